# revision 22
# baseline (speedup 1.0000x reference)
"""MinGRU Trainium2 kernel (B=8, T=8192, D=H=512), SPMD over 8 NeuronCores.

Strategy:
  - Data-parallel over batch: core b computes batch row b end-to-end.
  - Host pre-transposes x[b] -> xT [D, T] (bf16) so the matmul gets its
    contraction dim (d) on partitions with contiguous DMA, and the scan gets
    time on the free dimension. Output hT [H, T] f32 is transposed back on
    the host.
  - k = Wz @ x^T, th = Wh @ x^T as bf16 matmuls (1 cyc/row on the PE),
    accumulated over 4 d-subtiles into one PSUM bank per 128-channel h-group
    and 512-step time chunk.
  - ACT engine: a = sigmoid(-(k+bz)), s = sigmoid(th+bh) (bias/scale fused).
  - DVE: g~ = max(th+bh+0.5, s) (exact rewrite of the piecewise g(); one
    scalar_tensor_tensor), then a single HAND-AUTHORED fused-scan custom DVE
    op (MINGRU_FSCAN) computes
        h[t] = a[t]*h[t-1] + (1-a[t])*g~[t]
    in one instruction per (h-group, chunk): the (1-a)*g~ product runs on the
    forward pipeline stages while the 2-stage multiply-add recurrence uses the
    a-flop feedback path with a 1-cycle bubble (same rate as the stock
    tensor_tensor_scan, but no separate d1 pass, no third ACT pass, no
    GpSimd). Chunks chain via initial = previous chunk's last column.
"""

import os
import sys

import numpy as np

if "/opt/trn_rl_repo" not in sys.path:
    sys.path.insert(0, "/opt/trn_rl_repo")

P = 128
B, T, D, H = 8, 8192, 512, 512
GD, GH = D // P, H // P  # 4, 4
TC = 512  # time chunk; 1-bank PSUM tiles, 4 bufs per pool
NCORES = 8

_NC_CACHE = {}
LAST_RESULT = None  # BassKernelResults of the most recent run (for test.py)

_FSCAN_OP = None


def _fscan_reference(in0, in1, c0, c1, c2):
    """Numpy reference for CoreSim: h[t] = a[t]*h[t-1] + (1-a[t])*g[t]."""
    a = np.asarray(in0, np.float32)
    g = np.asarray(in1, np.float32)
    p = a.shape[0]
    fa = a.reshape(p, -1)
    fg = g.reshape(p, -1)
    if isinstance(c0, np.ndarray):
        state = c0.reshape(p).astype(np.float32).copy()
    else:
        state = np.full(p, float(c0), np.float32)
    out = np.empty_like(fa)
    for t in range(fa.shape[1]):
        state = fa[:, t] * state + (np.float32(1.0) - fa[:, t]) * fg[:, t]
        out[:, t] = state
    return out.reshape(a.shape)


def register_fscan():
    """Register the hand-authored MINGRU_FSCAN custom DVE op (TRN2/v3).

    Datapath per element (input lanes: 1=a, 2=g~, 3=1.0f, 4=init):
      blk0: t = 1 - a          (chains: 0 carries a, 1 carries g~, 3 init)
      blk1: u = t * g~
      blk2: m = a * state_prev (state from blk3's a-flop via NEXT_ALU_OUT_A;
                                first element reads init from chain 3)
            chain2 captures u from blk1's out-flop
      blk3: state = m + u      (writes out-flop + a-flop feedback)
      blk4-7: bypass; WR0 <- last block.
    FSM: uop0 = first element (init path) -> uop1 bubble (1 cycle, no
    consume, lets the a-flop result become visible) -> uop2 steady -> uop1 ...
    Each real element takes 2 cycles, same as the stock scan's rate.
    """
    global _FSCAN_OP
    if _FSCAN_OP is not None:
        return _FSCAN_OP

    from concourse.dve_ops import _SUB_OPCODE_FOR_NAME, CUSTOM_DVE_SPECS, OPS, DveOp
    from concourse.dve_spec import One, Spec, Src0, Src1
    from concourse.dve_uop import (
        ENABLE,
        AluInp,
        AluOp,
        DelayInp,
        DveOpSpec,
        InpSel,
        OutPath,
        OutSel,
        Trigger,
        UopConfig,
    )

    if "MINGRU_FSCAN" in _SUB_OPCODE_FOR_NAME:
        for op_ in OPS:
            if op_.name == "MINGRU_FSCAN":
                _FSCAN_OP = op_
                return op_

    # placeholder body (never lowered); reference drives CoreSim.
    spec = Spec(body=(One - Src0) * Src1, reference=_fscan_reference)

    def _compute_uop(first: bool) -> UopConfig:
        u = UopConfig()
        lanes = ((1, InpSel.SRC_0), (2, InpSel.SRC_1), (3, InpSel.ONE_F32), (4, InpSel.CONST_0))
        for lane, sel in lanes:
            u.inp[lane] = sel
            u.inp_enable[lane] = ENABLE
        dp = u.datapath_config
        # blk0: t = 1 - a; load/carry chains 0 (a), 1 (g~), 3 (init)
        dp[0].enable_alu(AluOp.SUBTRACT, AluInp.PREV_DELAY_2, AluInp.PREV_DELAY_0)
        dp[0].pass_through_delay(0, 1, 3)
        # blk1: u = t * g~; carry chains 0 (a), 3 (init)
        dp[1].enable_alu(AluOp.MULTIPLY, AluInp.PREV_ALU_OUT, AluInp.PREV_DELAY_1)
        dp[1].pass_through_delay(0, 3)
        # blk2: m = a * state_prev; capture u into chain 2
        state_src = AluInp.PREV_DELAY_3 if first else AluInp.NEXT_ALU_OUT_A
        dp[2].enable_alu(AluOp.MULTIPLY, AluInp.PREV_DELAY_0, state_src)
        dp[2].enable_delay_from_src(DelayInp.PREV_ALU_OUT, 2)
        # blk3: state = m + u; out-flop + a-flop (feedback)
        dp[3].enable_alu(AluOp.ADD, AluInp.PREV_ALU_OUT, AluInp.PREV_DELAY_2)
        dp[3].alu_out_a_enable = ENABLE
        for b in range(4, 8):
            dp[b].pass_through_alu()
        u.out[OutPath.WR0_LO] = OutSel.ALU_OUT
        u.out_enable[OutPath.WR0_LO] = ENABLE
        u.require_inp0 = ENABLE
        u.require_inp1 = ENABLE
        u.repeat_count = 1
        u.trigger = (Trigger.SRC_TENSOR_DONE, Trigger.COUNT, Trigger.NONE)
        u.next_uop = (0, 1, 0)  # done -> idle; else -> bubble
        return u

    def _bubble_uop() -> UopConfig:
        u = UopConfig()
        u.repeat_count = 1
        u.trigger = (Trigger.SRC_TENSOR_DONE, Trigger.COUNT, Trigger.NONE)
        u.next_uop = (0, 2, 0)  # done -> idle; else -> steady
        return u

    uops = [_compute_uop(first=True), _bubble_uop(), _compute_uop(first=False)]
    for u in uops:
        u.validate("v3")

    row = max(_SUB_OPCODE_FOR_NAME.values()) + 1
    assert row < 0x20

    class _HandDveOp(DveOp):
        def compile(self, ver):
            from concourse.dve_ops import _COMPILE_CACHE

            key = (self.name, ver)
            if key in _COMPILE_CACHE:
                return _COMPILE_CACHE[key]
            assert ver == "v3", "MINGRU_FSCAN is hand-authored for TRN2 (v3) only"
            r = DveOpSpec(name=self.name, opcode=row, uops=list(uops), rd1_en=True)
            _COMPILE_CACHE[key] = r
            return r

    op = _HandDveOp(name="MINGRU_FSCAN", spec=spec, subdim=False, uops_sha={})
    OPS.append(op)
    CUSTOM_DVE_SPECS[op.name] = spec
    _SUB_OPCODE_FOR_NAME[op.name] = row
    _FSCAN_OP = op
    return op


def _build_nc(t_len=T, tc=TC, mm="bf16", fscan=True):
    from contextlib import ExitStack

    import concourse.mybir as mybir
    import concourse.tile as tile
    from concourse import bacc

    f32 = mybir.dt.float32
    fmm = mybir.dt.bfloat16 if mm == "bf16" else mybir.dt.float32r
    Alu = mybir.AluOpType
    Act = mybir.ActivationFunctionType

    fscan_op = register_fscan() if fscan else None

    nchunk = t_len // tc
    nc = bacc.Bacc("TRN2", target_bir_lowering=False, debug=False)

    xT = nc.dram_tensor("xT", [D, t_len], fmm, kind="ExternalInput").ap()
    wzT = nc.dram_tensor("wzT", [D, H], fmm, kind="ExternalInput").ap()
    whT = nc.dram_tensor("whT", [D, H], fmm, kind="ExternalInput").ap()
    bzn = nc.dram_tensor("bzn", [P, GH], f32, kind="ExternalInput").ap()
    bzp = nc.dram_tensor("bzp", [P, GH], f32, kind="ExternalInput").ap()
    bhp = nc.dram_tensor("bhp", [P, GH], f32, kind="ExternalInput").ap()
    bh5 = nc.dram_tensor("bh5", [P, GH], f32, kind="ExternalInput").ap()
    hT = nc.dram_tensor("hT", [H, t_len], f32, kind="ExternalOutput").ap()

    xT_g = xT.rearrange("(g p) t -> p g t", p=P)
    hT_g = hT.rearrange("(g p) t -> p g t", p=P)

    with tile.TileContext(nc) as tctx, ExitStack() as ctx:
        singles = ctx.enter_context(tctx.tile_pool(name="singles", bufs=1))
        xpool = ctx.enter_context(tctx.tile_pool(name="xp", bufs=3))
        hpool = ctx.enter_context(tctx.tile_pool(name="hp", bufs=3))
        apool = ctx.enter_context(tctx.tile_pool(name="apool", bufs=5))
        spool = ctx.enter_context(tctx.tile_pool(name="spool", bufs=5))
        gpool = ctx.enter_context(tctx.tile_pool(name="gpool", bufs=5))
        dpool = ctx.enter_context(tctx.tile_pool(name="dpool", bufs=5))
        banks_per_tile = max(1, (tc * 4) // 2048)  # PSUM tiles pad to 1 bank
        psum_bufs = 4 // banks_per_tile  # 8 PSUM banks across k+t pools
        kp = ctx.enter_context(tctx.tile_pool(name="kp", bufs=psum_bufs, space="PSUM"))
        tp = ctx.enter_context(tctx.tile_pool(name="tp", bufs=psum_bufs, space="PSUM"))

        wzT_r = wzT.rearrange("(g p) h -> p g h", p=P)
        whT_r = whT.rearrange("(g p) h -> p g h", p=P)
        wz_sb = singles.tile([P, GD, H], fmm)
        wh_sb = singles.tile([P, GD, H], fmm)
        for g in range(GH):
            hs = slice(g * P, (g + 1) * P)
            nc.gpsimd.dma_start(out=wz_sb[:, :, hs], in_=wzT_r[:, :, hs])
            nc.gpsimd.dma_start(out=wh_sb[:, :, hs], in_=whT_r[:, :, hs])
        bzn_sb = singles.tile([P, GH], f32)
        nc.gpsimd.dma_start(out=bzn_sb, in_=bzn)
        bzp_sb = singles.tile([P, GH], f32)
        nc.gpsimd.dma_start(out=bzp_sb, in_=bzp)
        bhp_sb = singles.tile([P, GH], f32)
        nc.gpsimd.dma_start(out=bhp_sb, in_=bhp)
        bh5_sb = singles.tile([P, GH], f32)
        nc.gpsimd.dma_start(out=bh5_sb, in_=bh5)

        # Small first/last chunks: the pipeline fills sooner at the start
        # (less DMA/matmul ramp before the first scan) and drains sooner at
        # the end (the post-matmul ACT->gt->scan tail is shorter).
        if tc >= 1024 and t_len > 2 * tc:
            head = [tc // 4, tc // 4, tc // 2]
            tail = [tc // 2, tc // 4, tc // 4]
            mid = (t_len - sum(head) - sum(tail)) // tc
            chunk_sizes = head + [tc] * mid + tail
            assert sum(chunk_sizes) == t_len, (chunk_sizes, t_len)
        else:
            chunk_sizes = [tc] * nchunk

        h_prev = None
        prev_tc = None
        c_off = 0
        for c, tcc in enumerate(chunk_sizes):
            x_sb = xpool.tile([P, GD, tcc], fmm, tag="x")
            for gd in range(GD):
                nc.sync.dma_start(
                    out=x_sb[:, gd, :],
                    in_=xT_g[:, gd, c_off : c_off + tcc],
                )
            h_g_tiles = []
            for g in range(GH):
                kps = kp.tile([P, tcc], f32, tag="k")
                tps = tp.tile([P, tcc], f32, tag="t")
                nw = min(512, tcc)
                for ns in range(tcc // nw):
                    nsl = slice(ns * nw, (ns + 1) * nw)
                    for gd in range(GD):
                        nc.tensor.matmul(
                            kps[:, nsl],
                            wz_sb[:, gd, g * P : (g + 1) * P],
                            x_sb[:, gd, nsl],
                            start=(gd == 0),
                            stop=(gd == GD - 1),
                        )
                for ns in range(tcc // nw):
                    nsl = slice(ns * nw, (ns + 1) * nw)
                    for gd in range(GD):
                        nc.tensor.matmul(
                            tps[:, nsl],
                            wh_sb[:, gd, g * P : (g + 1) * P],
                            x_sb[:, gd, nsl],
                            start=(gd == 0),
                            stop=(gd == GD - 1),
                        )
                # s = sigmoid(th_mm + bh) -- first: g~ depends on it
                s_sb = spool.tile([P, tcc], f32, tag="s")
                nc.scalar.activation(
                    out=s_sb,
                    in_=tps,
                    func=Act.Sigmoid,
                    bias=bhp_sb[:, g : g + 1],
                    scale=1.0,
                )
                # a = sigmoid(-(k_mm + bz)) = Sigmoid(k_mm * -1 + (-bz))
                a_sb = apool.tile([P, tcc], f32, tag="a")
                nc.scalar.activation(
                    out=a_sb,
                    in_=kps,
                    func=Act.Sigmoid,
                    bias=bzn_sb[:, g : g + 1],
                    scale=-1.0,
                )
                # g~ = max(th_mm + (bh+0.5), s)
                g_sb = gpool.tile([P, tcc], f32, tag="g")
                nc.vector.scalar_tensor_tensor(
                    out=g_sb,
                    in0=tps,
                    scalar=bh5_sb[:, g : g + 1],
                    in1=s_sb,
                    op0=Alu.add,
                    op1=Alu.max,
                )
                h_sb = hpool.tile([P, tcc], f32, tag=f"h{g}")
                init = 0.0 if c == 0 else h_prev[g][:, prev_tc - 1 : prev_tc]
                if fscan:
                    # h[t] = a[t]*h[t-1] + (1-a[t])*g~[t], fused in one op
                    nc.vector._custom_dve(
                        fscan_op, out=h_sb, in0=a_sb, in1=g_sb, s0=init
                    )
                else:
                    # stock path: z = sigmoid(k+bz); d1 = z*g~ on GpSimd;
                    # scan computes y = -h, negated on the host
                    z_sb = spool.tile([P, tcc], f32, tag="z")
                    nc.scalar.activation(
                        out=z_sb,
                        in_=kps,
                        func=Act.Sigmoid,
                        bias=bzp_sb[:, g : g + 1],
                        scale=1.0,
                    )
                    d_sb = dpool.tile([P, tcc], f32, tag="d")
                    nc.gpsimd.tensor_tensor(out=d_sb, in0=z_sb, in1=g_sb, op=Alu.mult)
                    nc.vector.tensor_tensor_scan(
                        out=h_sb,
                        data0=a_sb,
                        data1=d_sb,
                        initial=init,
                        op0=Alu.mult,
                        op1=Alu.subtract,
                    )
                nc.sync.dma_start(
                    out=hT_g[:, g, c_off : c_off + tcc], in_=h_sb
                )
                h_g_tiles.append(h_sb)
            h_prev = h_g_tiles
            prev_tc = tcc
            c_off += tcc
    nc.compile()
    return nc


def get_nc(t_len=T, tc=TC, mm="bf16", fscan=True):
    key = (t_len, tc, mm, fscan)
    if key not in _NC_CACHE:
        _NC_CACHE[key] = _build_nc(t_len, tc, mm, fscan)
    return _NC_CACHE[key]


def _mm_np_dtype(mm="bf16"):
    if mm == "bf16":
        import ml_dtypes

        return np.dtype(ml_dtypes.bfloat16)
    return np.dtype(np.float32)


def _prep_shared(Wz, bz, Wh, bh, mm="bf16"):
    f = np.float32
    fm = _mm_np_dtype(mm)
    return {
        "wzT": np.ascontiguousarray(Wz.T).astype(fm),
        "whT": np.ascontiguousarray(Wh.T).astype(fm),
        "bzn": np.ascontiguousarray((-bz).reshape(GH, P).T, dtype=f),
        "bzp": np.ascontiguousarray(bz.reshape(GH, P).T, dtype=f),
        "bhp": np.ascontiguousarray(bh.reshape(GH, P).T, dtype=f),
        "bh5": np.ascontiguousarray((bh + 0.5).reshape(GH, P).T, dtype=f),
    }


def kernel(x, Wz, bz, Wh, bh):
    global LAST_RESULT
    from concourse import bass_utils

    x = np.asarray(x, dtype=np.float32)
    assert x.shape == (B, T, D), x.shape

    mm = os.environ.get("MINGRU_MM", "bf16")
    tc = int(os.environ.get("MINGRU_TC", str(TC)))
    fscan = os.environ.get("MINGRU_FSCAN", "1") == "1"
    nc = get_nc(tc=tc, mm=mm, fscan=fscan)
    fm = _mm_np_dtype(mm)
    shared = _prep_shared(
        np.asarray(Wz, np.float32),
        np.asarray(bz, np.float32),
        np.asarray(Wh, np.float32),
        np.asarray(bh, np.float32),
        mm=mm,
    )
    in_maps = []
    for b in range(NCORES):
        m = {"xT": np.ascontiguousarray(x[b].T).astype(fm)}
        m.update(shared)
        in_maps.append(m)

    res = bass_utils.run_bass_kernel_spmd(
        nc,
        in_maps,
        core_ids=list(range(NCORES)),
        trace=os.environ.get("MINGRU_TRACE", "0") == "1",
    )
    LAST_RESULT = res
    out = np.stack([res.results[b]["hT"].T for b in range(NCORES)])
    if not fscan:
        # the stock-scan path computes y = -h
        out = np.negative(out, out=out)
    return np.ascontiguousarray(out, dtype=np.float32)
